# revision 30
# baseline (speedup 1.0000x reference)
"""Trainium2 Bass kernel for CamPredModule (moe_routing) on 8 NeuronCores.

Reference semantics (eval path):
    ip        = one_hot(init_prob)                      # [B,N]
    init_feat = max(feat[b, ip_b], 0)                   # masked max over N
    ce        = layer_norm(cam_emb[ip_b])               # [B,N]
    cf        = relu(spatial_max(feat[b, ip_b]))        # [B,C]
    h         = relu(relu(cf@W1.T+b1)@W2.T+b2)
    cp        = layer_norm(h@Wp.T)/10
    sel       = argmax over candidates of (cp+ce)       # one-hot [B,N]
    overall   = max(init_feat, feat[b, sel_b])
    returns (overall, ce, cp, sel_onehot)

Sharding: core k handles (b = k//4, spatial chunk q = k%4 of H).  Each core
only touches the two needed camera slices (init + selected): the init slice
is host-sharded (a pure gather by the init_prob input index), the selected
slice is fetched with a dynamic-offset DMA using the on-device routing
result.

Two launches (the ncfw collective path has a ~60us fixed barrier on this
runtime, so the cross-core max is relayed through the host instead —
pure gather/concat of tiny [1,128] vectors, no host arithmetic):
  launch 1 (raw Bass, SP+DVE+PE): per-core spatial max of the init
      chunk, PE-transposed and stored as a contiguous row     (~32us)
  launch 2 (Tile): combine the 8 maxes, replicated router MLP for
      both batches, dynamic-offset gather of the selected camera,
      elementwise max against the relu'd init chunk, store    (~58us)
Per-core HBM traffic: launch 1 reads 5.5MB; launch 2 moves 16.5MB
(init re-read + gather + store) at the ~358GB/s/core roofline.
"""

import numpy as np

B, N, C, H, W = 2, 8, 128, 120, 360
HW = H * W          # 43200
NCORES = 8
CPB = 4             # cores (spatial chunks) per batch
F = HW // CPB       # 10800 elements per chunk per channel
NT = 3              # DMA/compute sub-tiles per chunk
F_TILE = F // NT    # 3600
HC = H // CPB       # 30 rows of H per chunk
LN_EPS = 1e-5

_built1 = None
_built2 = None


def _build1():
    """Launch 1: spatial max of the (host-gathered) init-camera chunk.

    Raw Bass (no TileContext): only the SP + DVE engines do work, manual
    semaphores, and a light tail — the Tile drain/cleanup epilogue costs
    ~15us, which matters at this kernel's ~25us scale.
    """
    import concourse.bass as bass
    import concourse.mybir as mybir

    f32 = mybir.dt.float32
    X = mybir.AxisListType.X
    # descending tile sizes: the last tile's reduce tail is off the DMA
    # critical path sooner
    BNDS1 = [0, 3600, 7200, 9900, 10800]
    NT1 = len(BNDS1) - 1

    nc = bass.Bass("TRN2", target_bir_lowering=False, debug=False, num_devices=NCORES)
    feat_init = nc.dram_tensor("feat_init", [C, F], f32, kind="ExternalInput").ap()
    ident = nc.dram_tensor("ident", [C, C], f32, kind="ExternalInput").ap()
    # stored as a row: a [C,1] column store scatters 128 4-byte descriptors
    # (~8.5us); transposing on the PE and storing [1,C] contiguous is ~2us.
    lmax_out = nc.dram_tensor("lmax_out", [1, C], f32, kind="ExternalOutput").ap()

    # one semaphore per in-flight DMA: a shared counter is unsound because
    # SDMA engines drain their per-engine rings independently (a later DMA's
    # increments can land before an earlier DMA fully completes).
    dsems = [nc.alloc_semaphore(name=f"dsem{t}") for t in range(NT1)]
    isem = nc.alloc_semaphore(name="isem")
    ssem = nc.alloc_semaphore(name="ssem")
    vsem = nc.alloc_semaphore(name="vsem")
    psem = nc.alloc_semaphore(name="psem")
    msem = nc.alloc_semaphore(name="msem")
    tiles = [
        nc.alloc_sbuf_tensor(f"a{t}", [C, BNDS1[t + 1] - BNDS1[t]], f32).ap()
        for t in range(NT1)
    ]
    ident_sb = nc.alloc_sbuf_tensor("ident_sb", [C, C], f32).ap()
    pmax = nc.alloc_sbuf_tensor("pmax", [C, NT1], f32).ap()
    lmax = nc.alloc_sbuf_tensor("lmax", [C, 1], f32).ap()
    lrow = nc.alloc_sbuf_tensor("lrow", [1, C], f32).ap()
    lrow_ps = nc.alloc_psum_tensor("lrow_ps", [1, C], f32).ap()

    with nc.Block(no_gpsimd_drain=True) as block:

        @block.sync
        def _(sync):
            for t in range(NT1):
                sync.dma_start(
                    tiles[t], feat_init[:, BNDS1[t] : BNDS1[t + 1]]
                ).then_inc(dsems[t], 16)
            # needed only by the PE transpose at the very end
            sync.dma_start(ident_sb, ident).then_inc(isem, 16)
            sync.wait_ge(vsem, 2)
            sync.dma_start(lmax_out, lrow).then_inc(ssem, 16)
            sync.wait_ge(ssem, 16)

        @block.vector
        def _(vector):
            # DVE has no same-engine RAW interlock between back-to-back
            # instructions; the final reduce must wait for the partials'
            # writebacks via a self-semaphore.
            for t in range(NT1):
                vector.wait_ge(dsems[t], 16)
                vector.reduce_max(
                    out=pmax[:, t : t + 1], in_=tiles[t], axis=X
                ).then_inc(psem, 1)
            vector.wait_ge(psem, NT1)
            vector.reduce_max(out=lmax, in_=pmax, axis=X).then_inc(vsem, 1)
            vector.wait_ge(msem, 1)
            vector.tensor_copy(out=lrow, in_=lrow_ps).then_inc(vsem, 1)

        @block.tensor
        def _(tensor):
            tensor.wait_ge(isem, 16)
            tensor.wait_ge(vsem, 1)
            nc.tensor.matmul(
                out=lrow_ps, lhsT=lmax, rhs=ident_sb, start=True, stop=True
            ).then_inc(msem, 1)

    # reset semaphores so repeated executions of this NEFF start clean
    all_sems = sorted(s.num for s in (*dsems, isem, ssem, vsem, psem, msem))
    nc.gpsimd.sem_clear(range(min(all_sems), max(all_sems) + 1))
    return nc


def _build2():
    """Launch 2: combine maxes, router, dynamic gather, combine, store."""
    import concourse.bacc as bacc
    import concourse.bass as bass
    import concourse.mybir as mybir
    import concourse.tile as tile

    f32 = mybir.dt.float32
    i32 = mybir.dt.int32
    X = mybir.AxisListType.X
    Relu = mybir.ActivationFunctionType.Relu
    Sqrt = mybir.ActivationFunctionType.Sqrt
    AT = mybir.AluOpType

    nc = bacc.Bacc("TRN2", target_bir_lowering=False, debug=False, num_devices=NCORES)

    feat_blk = nc.dram_tensor("feat_blk", [N * C, F], f32, kind="ExternalInput").ap()
    feat_init = nc.dram_tensor("feat_init", [C, F], f32, kind="ExternalInput").ap()
    # packed small inputs (fewer DMAs -> router starts sooner):
    #   big128 columns: w1t | w2t | wpt | b1c | b2c | lmax8
    #   pk8 columns:    cam_emb | ipf2
    #   pk2 columns:    cand2 | maskneg2 | nidx2
    W128 = C + C + N + 1 + 1 + NCORES
    big128 = nc.dram_tensor("big128", [C, W128], f32, kind="ExternalInput").ap()
    pk8 = nc.dram_tensor("pk8", [N, N + B], f32, kind="ExternalInput").ap()
    pk2 = nc.dram_tensor("pk2", [B, 3 * N], f32, kind="ExternalInput").ap()

    out_blk = nc.dram_tensor("out_blk", [C, F], f32, kind="ExternalOutput").ap()
    ce_out = nc.dram_tensor("ce_out", [B, N], f32, kind="ExternalOutput").ap()
    cp_out = nc.dram_tensor("cp_out", [B, N], f32, kind="ExternalOutput").ap()
    sel_out = nc.dram_tensor("sel_out", [B, N], f32, kind="ExternalOutput").ap()

    with tile.TileContext(nc) as tc:
        with (
            tc.tile_pool(name="big", bufs=1) as bigp,
            tc.tile_pool(name="small", bufs=1) as sp,
            tc.tile_pool(name="selp", bufs=2) as selp,
            tc.tile_pool(name="psum", bufs=1, space="PSUM") as pp,
        ):
            # ---- packed small loads (scalar HWDGE ring)
            big_sb = sp.tile([C, W128], f32)
            nc.scalar.dma_start(big_sb, big128)
            pk8_sb = sp.tile([N, N + B], f32)
            nc.scalar.dma_start(pk8_sb, pk8)
            pk2_sb = sp.tile([B, 3 * N], f32)
            nc.scalar.dma_start(pk2_sb, pk2)
            w1_sb = big_sb[:, 0:C]
            w2_sb = big_sb[:, C : 2 * C]
            wp_sb = big_sb[:, 2 * C : 2 * C + N]
            b1_sb = big_sb[:, 2 * C + N : 2 * C + N + 1]
            b2_sb = big_sb[:, 2 * C + N + 1 : 2 * C + N + 2]
            lmax8_sb = big_sb[:, 2 * C + N + 2 : W128]
            ce_mat = pk8_sb[:, 0:N]
            ipf2_sb = pk8_sb[:, N : N + B]
            cand2_sb = pk2_sb[:, 0:N]
            maskneg2_sb = pk2_sb[:, N : 2 * N]
            nidx2_sb = pk2_sb[:, 2 * N : 3 * N]

            # ---- re-load the init chunk; clamp at 0 on the otherwise-idle
            # ACT engine while the router runs (overall = max(relu(init), sel)).
            # Descending tile sizes: the final gather->max->store chain in
            # phase D is short, trimming the serial tail after the last DMA.
            BNDS = [0, 3600, 7200, 9900, 10800]
            NTD = len(BNDS) - 1
            accs = []
            for t in range(NTD):
                sl = slice(BNDS[t], BNDS[t + 1])
                a = bigp.tile(
                    [C, BNDS[t + 1] - BNDS[t]], f32, name=f"acc{t}", tag=f"acc{t}"
                )
                accs.append(a)
                nc.sync.dma_start(a, feat_init[:, sl])
                nc.scalar.activation(a, a, Relu)

            # ---- global per-batch spatial max: columns 0-3 are batch 0's
            # chunks, 4-7 batch 1's (fixed core->(b,q) mapping)
            cf2 = sp.tile([C, B], f32)
            nc.vector.reduce_max(out=cf2[:, 0:1], in_=lmax8_sb[:, 0:CPB], axis=X)
            nc.vector.reduce_max(out=cf2[:, 1:2], in_=lmax8_sb[:, CPB:NCORES], axis=X)
            nc.vector.tensor_scalar_max(cf2, cf2, 0.0)

            # ---- router MLP for both batches (column layout [C, 2]);
            # bias+relu folded into one DVE tensor_scalar per layer
            h1p = pp.tile([C, B], f32)
            nc.tensor.matmul(out=h1p, lhsT=w1_sb, rhs=cf2, start=True, stop=True)
            h1 = sp.tile([C, B], f32)
            nc.vector.tensor_scalar(h1, h1p, b1_sb, 0.0, AT.add, AT.max)
            h2p = pp.tile([C, B], f32)
            nc.tensor.matmul(out=h2p, lhsT=w2_sb, rhs=h1, start=True, stop=True)
            h2 = sp.tile([C, B], f32)
            nc.vector.tensor_scalar(h2, h2p, b2_sb, 0.0, AT.add, AT.max)

            cp_pre = pp.tile([B, N], f32)
            nc.tensor.matmul(out=cp_pre, lhsT=h2, rhs=wp_sb, start=True, stop=True)
            ce_pre = pp.tile([B, N], f32)
            nc.tensor.matmul(out=ce_pre, lhsT=ipf2_sb, rhs=ce_mat, start=True, stop=True)

            eps2 = sp.tile([B, 1], f32)
            nc.vector.memset(eps2, LN_EPS)

            # ---- LayerNorm over the free axis of a [2, N] tile.
            # DVE-centric: single ACT visit (the sqrt), everything else DVE.
            def ln_rows(pre, post_scale, nm):
                pre_sb = sp.tile([B, N], f32, name=f"ln_pre_{nm}")
                nc.vector.tensor_copy(out=pre_sb, in_=pre)
                s = sp.tile([B, 1], f32, name=f"ln_s_{nm}")
                nc.vector.reduce_sum(out=s, in_=pre_sb, axis=X)
                sq = sp.tile([B, N], f32, name=f"ln_sq_{nm}")
                nc.vector.tensor_tensor(out=sq, in0=pre_sb, in1=pre_sb, op=AT.mult)
                s2 = sp.tile([B, 1], f32, name=f"ln_s2_{nm}")
                nc.vector.reduce_sum(out=s2, in_=sq, axis=X)
                m = sp.tile([B, 1], f32, name=f"ln_m_{nm}")
                nc.vector.tensor_scalar(m, s, 1.0 / N, None, AT.mult)
                xc = sp.tile([B, N], f32, name=f"ln_xc_{nm}")
                nc.vector.tensor_scalar(xc, pre_sb, m[:, 0:1], None, AT.subtract)
                m2 = sp.tile([B, 1], f32, name=f"ln_m2_{nm}")
                nc.vector.tensor_tensor(out=m2, in0=m, in1=m, op=AT.mult)
                v = sp.tile([B, 1], f32, name=f"ln_v_{nm}")
                nc.vector.tensor_scalar(v, s2, 1.0 / N, None, AT.mult)
                nc.vector.tensor_tensor(out=v, in0=v, in1=m2, op=AT.subtract)
                sd = sp.tile([B, 1], f32, name=f"ln_sd_{nm}")
                nc.scalar.activation(sd, v, Sqrt, bias=eps2[:, 0:1])
                rs = sp.tile([B, 1], f32, name=f"ln_rs_{nm}")
                nc.vector.reciprocal(rs, sd)
                if post_scale != 1.0:
                    nc.vector.tensor_scalar(rs, rs, post_scale, None, AT.mult)
                out = sp.tile([B, N], f32, name=f"ln_out_{nm}")
                nc.vector.tensor_scalar(out, xc, rs[:, 0:1], None, AT.mult)
                return out

            cp_row = ln_rows(cp_pre, 0.1, "cp")
            ce_row = ln_rows(ce_pre, 1.0, "ce")
            nc.scalar.dma_start(cp_out, cp_row)
            nc.scalar.dma_start(ce_out, ce_row)

            # ---- masked argmax -> one-hot selection for both batches
            logits = sp.tile([B, N], f32)
            nc.vector.tensor_add(logits, cp_row, ce_row)
            ml = sp.tile([B, N], f32)
            nc.vector.tensor_mul(ml, logits, cand2_sb)
            nc.vector.tensor_add(ml, ml, maskneg2_sb)
            mx2 = sp.tile([B, 1], f32)
            nc.vector.reduce_max(out=mx2, in_=ml, axis=X)
            sel2 = sp.tile([B, N], f32)
            nc.vector.tensor_scalar(sel2, ml, mx2[:, 0:1], None, AT.is_equal)
            nc.scalar.dma_start(sel_out, sel2)

            # ---- this core's selected camera index: nidx2 is host-masked to
            # this core's batch row, so a sum over both rows yields it.
            tsel = sp.tile([B, N], f32)
            nc.vector.tensor_mul(tsel, sel2, nidx2_sb)
            selv2 = sp.tile([B, 1], f32)
            nc.vector.reduce_sum(out=selv2, in_=tsel, axis=X)
            ones2 = sp.tile([B, 1], f32)
            nc.vector.memset(ones2, 1.0)
            svp = pp.tile([1, 1], f32)
            nc.tensor.matmul(out=svp, lhsT=selv2, rhs=ones2, start=True, stop=True)
            svi = sp.tile([1, 1], i32)
            nc.vector.tensor_copy(out=svi, in_=svp)
            r_sel = nc.values_load(
                svi[0:1, 0:1],
                engines=(mybir.EngineType.SP,),
                min_val=0,
                max_val=N - 1,
                skip_runtime_bounds_check=True,
            )

            # ---- gather selected camera (sync ring), combine (DVE),
            #      store (scalar ring)
            for t in range(NTD):
                sl = slice(BNDS[t], BNDS[t + 1])
                w = BNDS[t + 1] - BNDS[t]
                st = selp.tile([C, w], f32, name=f"selt{t}", tag="selt")
                nc.sync.dma_start(st, feat_blk[bass.ds(r_sel * C, C), sl])
                nc.vector.tensor_tensor(out=accs[t], in0=accs[t], in1=st, op=AT.max)
                nc.scalar.dma_start(out_blk[:, sl], accs[t])

    nc.compile()
    return nc


LAST_RESULTS = None
LAST_EXEC_NS = None


def kernel(**inputs):
    global _built1, _built2, LAST_RESULTS, LAST_EXEC_NS
    from concourse import bass_utils

    feat = np.ascontiguousarray(np.asarray(inputs["feat"], dtype=np.float32))
    init_prob = np.asarray(inputs["init_prob"]).astype(np.int64)
    keep_cams = np.asarray(inputs["keep_cams"])
    cam_emb = np.ascontiguousarray(np.asarray(inputs["cam_emb"], np.float32))
    W1 = np.asarray(inputs["W1"], np.float32)
    b1 = np.asarray(inputs["b1"], np.float32)
    W2 = np.asarray(inputs["W2"], np.float32)
    b2 = np.asarray(inputs["b2"], np.float32)
    Wp = np.asarray(inputs["Wp"], np.float32)

    if _built1 is None:
        _built1 = _build1()
    if _built2 is None:
        _built2 = _build2()

    fr = feat.reshape(B, N * C, HW)
    eye = np.eye(N, dtype=np.float32)
    ipf = eye[init_prob]                                 # [B, N]
    cand = (1.0 - ipf) * keep_cams.astype(np.float32)    # [B, N]

    blks = []
    inits = []
    for k in range(NCORES):
        b, q = divmod(k, CPB)
        ip = int(init_prob[b])
        blk = np.ascontiguousarray(fr[b][:, q * F : (q + 1) * F])
        blks.append(blk)
        inits.append(np.ascontiguousarray(blk[ip * C : (ip + 1) * C]))

    # ---- launch 1: per-core spatial max of the init chunk
    ident = np.eye(C, dtype=np.float32)
    in_maps1 = [{"feat_init": inits[k], "ident": ident} for k in range(NCORES)]
    res1 = bass_utils.run_bass_kernel_spmd(
        _built1, in_maps1, core_ids=list(range(NCORES))
    )

    # host relay (pure concatenation/layout, no arithmetic)
    lmax8 = np.ascontiguousarray(
        np.concatenate(
            [res1.results[k]["lmax_out"] for k in range(NCORES)], axis=0
        ).T
    )

    # ---- launch 2 (packed small inputs)
    big128 = np.ascontiguousarray(
        np.concatenate(
            [W1.T, W2.T, Wp.T, b1[:, None], b2[:, None], lmax8], axis=1
        ).astype(np.float32)
    )
    pk8 = np.ascontiguousarray(
        np.concatenate([cam_emb, ipf.T], axis=1).astype(np.float32)
    )
    maskneg = (cand - 1.0) * 1.0e30
    in_maps2 = []
    for k in range(NCORES):
        b, q = divmod(k, CPB)
        nidx2b = np.zeros((B, N), np.float32)
        nidx2b[b] = np.arange(N, dtype=np.float32)
        pk2 = np.ascontiguousarray(
            np.concatenate([cand, maskneg, nidx2b], axis=1).astype(np.float32)
        )
        in_maps2.append(
            {
                "big128": big128,
                "pk8": pk8,
                "pk2": pk2,
                "feat_blk": blks[k],
                "feat_init": inits[k],
            }
        )
    res2 = bass_utils.run_bass_kernel_spmd(
        _built2, in_maps2, core_ids=list(range(NCORES))
    )
    LAST_RESULTS = (res1, res2)
    if res1.exec_time_ns is not None and res2.exec_time_ns is not None:
        LAST_EXEC_NS = res1.exec_time_ns + res2.exec_time_ns
    outs = res2.results

    overall = np.empty((B, C, H, W), np.float32)
    for k in range(NCORES):
        b, q = divmod(k, CPB)
        overall[b, :, q * HC : (q + 1) * HC, :] = outs[k]["out_blk"].reshape(C, HC, W)
    ce = outs[0]["ce_out"]
    cp = outs[0]["cp_out"]
    sel = outs[0]["sel_out"]
    return overall, ce, cp, sel


# revision 31
# speedup vs baseline: 1.0246x; 1.0246x over previous
"""Trainium2 Bass kernel for CamPredModule (moe_routing) on 8 NeuronCores.

Reference semantics (eval path):
    ip        = one_hot(init_prob)                      # [B,N]
    init_feat = max(feat[b, ip_b], 0)                   # masked max over N
    ce        = layer_norm(cam_emb[ip_b])               # [B,N]
    cf        = relu(spatial_max(feat[b, ip_b]))        # [B,C]
    h         = relu(relu(cf@W1.T+b1)@W2.T+b2)
    cp        = layer_norm(h@Wp.T)/10
    sel       = argmax over candidates of (cp+ce)       # one-hot [B,N]
    overall   = max(init_feat, feat[b, sel_b])
    returns (overall, ce, cp, sel_onehot)

Sharding: core k handles (b = k//4, spatial chunk q = k%4 of H).  Each core
only touches the two needed camera slices (init + selected): the init slice
is host-sharded (a pure gather by the init_prob input index), the selected
slice is fetched with a dynamic-offset DMA using the on-device routing
result.

Two launches (the ncfw collective path has a ~60us fixed barrier on this
runtime, so the cross-core max is relayed through the host instead —
pure gather/concat of tiny [1,128] vectors, no host arithmetic):
  launch 1 (raw Bass, SP+DVE+PE): per-core spatial max of the init
      chunk, PE-transposed and stored as a contiguous row     (~32us)
  launch 2 (Tile): combine the 8 maxes, replicated router MLP for
      both batches, dynamic-offset gather of the selected camera,
      elementwise max against the relu'd init chunk, store    (~58us)
Per-core HBM traffic: launch 1 reads 5.5MB; launch 2 moves 16.5MB
(init re-read + gather + store) at the ~358GB/s/core roofline.
"""

import numpy as np

B, N, C, H, W = 2, 8, 128, 120, 360
HW = H * W          # 43200
NCORES = 8
CPB = 4             # cores (spatial chunks) per batch
F = HW // CPB       # 10800 elements per chunk per channel
NT = 3              # DMA/compute sub-tiles per chunk
F_TILE = F // NT    # 3600
HC = H // CPB       # 30 rows of H per chunk
LN_EPS = 1e-5

_built1 = None
_built2 = None


def _build1():
    """Launch 1: spatial max of the (host-gathered) init-camera chunk.

    Raw Bass (no TileContext): only the SP + DVE engines do work, manual
    semaphores, and a light tail — the Tile drain/cleanup epilogue costs
    ~15us, which matters at this kernel's ~25us scale.
    """
    import concourse.bass as bass
    import concourse.mybir as mybir

    f32 = mybir.dt.float32
    X = mybir.AxisListType.X
    # small first tile so the DVE reduce chain starts as early as possible;
    # the serial reduce work (~12us) then hides almost fully under the DMA
    BNDS1 = [0, 900, 4500, 8100, 10800]
    NT1 = len(BNDS1) - 1

    nc = bass.Bass("TRN2", target_bir_lowering=False, debug=False, num_devices=NCORES)
    feat_init = nc.dram_tensor("feat_init", [C, F], f32, kind="ExternalInput").ap()
    ident = nc.dram_tensor("ident", [C, C], f32, kind="ExternalInput").ap()
    # stored as a row: a [C,1] column store scatters 128 4-byte descriptors
    # (~8.5us); transposing on the PE and storing [1,C] contiguous is ~2us.
    lmax_out = nc.dram_tensor("lmax_out", [1, C], f32, kind="ExternalOutput").ap()

    # one semaphore per in-flight DMA: a shared counter is unsound because
    # SDMA engines drain their per-engine rings independently (a later DMA's
    # increments can land before an earlier DMA fully completes).
    dsems = [nc.alloc_semaphore(name=f"dsem{t}") for t in range(NT1)]
    isem = nc.alloc_semaphore(name="isem")
    ssem = nc.alloc_semaphore(name="ssem")
    vsem = nc.alloc_semaphore(name="vsem")
    psem = nc.alloc_semaphore(name="psem")
    msem = nc.alloc_semaphore(name="msem")
    tiles = [
        nc.alloc_sbuf_tensor(f"a{t}", [C, BNDS1[t + 1] - BNDS1[t]], f32).ap()
        for t in range(NT1)
    ]
    ident_sb = nc.alloc_sbuf_tensor("ident_sb", [C, C], f32).ap()
    pmax = nc.alloc_sbuf_tensor("pmax", [C, NT1], f32).ap()
    lmax = nc.alloc_sbuf_tensor("lmax", [C, 1], f32).ap()
    lrow = nc.alloc_sbuf_tensor("lrow", [1, C], f32).ap()
    lrow_ps = nc.alloc_psum_tensor("lrow_ps", [1, C], f32).ap()

    with nc.Block(no_gpsimd_drain=True) as block:

        @block.sync
        def _(sync):
            for t in range(NT1):
                sync.dma_start(
                    tiles[t], feat_init[:, BNDS1[t] : BNDS1[t + 1]]
                ).then_inc(dsems[t], 16)
            # needed only by the PE transpose at the very end
            sync.dma_start(ident_sb, ident).then_inc(isem, 16)
            sync.wait_ge(vsem, 2)
            sync.dma_start(lmax_out, lrow).then_inc(ssem, 16)
            sync.wait_ge(ssem, 16)

        @block.vector
        def _(vector):
            # DVE has no same-engine RAW interlock between back-to-back
            # instructions; the final reduce must wait for the partials'
            # writebacks via a self-semaphore.
            for t in range(NT1):
                vector.wait_ge(dsems[t], 16)
                vector.reduce_max(
                    out=pmax[:, t : t + 1], in_=tiles[t], axis=X
                ).then_inc(psem, 1)
            vector.wait_ge(psem, NT1)
            vector.reduce_max(out=lmax, in_=pmax, axis=X).then_inc(vsem, 1)
            vector.wait_ge(msem, 1)
            vector.tensor_copy(out=lrow, in_=lrow_ps).then_inc(vsem, 1)

        @block.tensor
        def _(tensor):
            tensor.wait_ge(isem, 16)
            tensor.wait_ge(vsem, 1)
            nc.tensor.matmul(
                out=lrow_ps, lhsT=lmax, rhs=ident_sb, start=True, stop=True
            ).then_inc(msem, 1)

    # reset semaphores so repeated executions of this NEFF start clean
    all_sems = sorted(s.num for s in (*dsems, isem, ssem, vsem, psem, msem))
    nc.gpsimd.sem_clear(range(min(all_sems), max(all_sems) + 1))
    return nc


def _build2():
    """Launch 2: combine maxes, router, dynamic gather, combine, store."""
    import concourse.bacc as bacc
    import concourse.bass as bass
    import concourse.mybir as mybir
    import concourse.tile as tile

    f32 = mybir.dt.float32
    i32 = mybir.dt.int32
    X = mybir.AxisListType.X
    Relu = mybir.ActivationFunctionType.Relu
    Sqrt = mybir.ActivationFunctionType.Sqrt
    AT = mybir.AluOpType

    nc = bacc.Bacc("TRN2", target_bir_lowering=False, debug=False, num_devices=NCORES)

    feat_blk = nc.dram_tensor("feat_blk", [N * C, F], f32, kind="ExternalInput").ap()
    feat_init = nc.dram_tensor("feat_init", [C, F], f32, kind="ExternalInput").ap()
    # packed small inputs (fewer DMAs -> router starts sooner):
    #   big128 columns: w1t | w2t | wpt | b1c | b2c | lmax8
    #   pk8 columns:    cam_emb | ipf2
    #   pk2 columns:    cand2 | maskneg2 | nidx2
    W128 = C + C + N + 1 + 1 + NCORES
    big128 = nc.dram_tensor("big128", [C, W128], f32, kind="ExternalInput").ap()
    pk8 = nc.dram_tensor("pk8", [N, N + B], f32, kind="ExternalInput").ap()
    pk2 = nc.dram_tensor("pk2", [B, 3 * N], f32, kind="ExternalInput").ap()

    out_blk = nc.dram_tensor("out_blk", [C, F], f32, kind="ExternalOutput").ap()
    ce_out = nc.dram_tensor("ce_out", [B, N], f32, kind="ExternalOutput").ap()
    cp_out = nc.dram_tensor("cp_out", [B, N], f32, kind="ExternalOutput").ap()
    sel_out = nc.dram_tensor("sel_out", [B, N], f32, kind="ExternalOutput").ap()

    with tile.TileContext(nc) as tc:
        with (
            tc.tile_pool(name="big", bufs=1) as bigp,
            tc.tile_pool(name="small", bufs=1) as sp,
            tc.tile_pool(name="selp", bufs=2) as selp,
            tc.tile_pool(name="psum", bufs=1, space="PSUM") as pp,
        ):
            # ---- packed small loads (scalar HWDGE ring)
            big_sb = sp.tile([C, W128], f32)
            nc.scalar.dma_start(big_sb, big128)
            pk8_sb = sp.tile([N, N + B], f32)
            nc.scalar.dma_start(pk8_sb, pk8)
            pk2_sb = sp.tile([B, 3 * N], f32)
            nc.scalar.dma_start(pk2_sb, pk2)
            w1_sb = big_sb[:, 0:C]
            w2_sb = big_sb[:, C : 2 * C]
            wp_sb = big_sb[:, 2 * C : 2 * C + N]
            b1_sb = big_sb[:, 2 * C + N : 2 * C + N + 1]
            b2_sb = big_sb[:, 2 * C + N + 1 : 2 * C + N + 2]
            lmax8_sb = big_sb[:, 2 * C + N + 2 : W128]
            ce_mat = pk8_sb[:, 0:N]
            ipf2_sb = pk8_sb[:, N : N + B]
            cand2_sb = pk2_sb[:, 0:N]
            maskneg2_sb = pk2_sb[:, N : 2 * N]
            nidx2_sb = pk2_sb[:, 2 * N : 3 * N]

            # ---- re-load the init chunk; clamp at 0 on the otherwise-idle
            # ACT engine while the router runs (overall = max(relu(init), sel)).
            # Descending tile sizes: the final gather->max->store chain in
            # phase D is short, trimming the serial tail after the last DMA.
            BNDS = [0, 3600, 7200, 9900, 10800]
            NTD = len(BNDS) - 1
            accs = []
            for t in range(NTD):
                sl = slice(BNDS[t], BNDS[t + 1])
                a = bigp.tile(
                    [C, BNDS[t + 1] - BNDS[t]], f32, name=f"acc{t}", tag=f"acc{t}"
                )
                accs.append(a)
                nc.sync.dma_start(a, feat_init[:, sl])
                nc.scalar.activation(a, a, Relu)

            # ---- global per-batch spatial max: columns 0-3 are batch 0's
            # chunks, 4-7 batch 1's (fixed core->(b,q) mapping)
            cf2 = sp.tile([C, B], f32)
            nc.vector.reduce_max(out=cf2[:, 0:1], in_=lmax8_sb[:, 0:CPB], axis=X)
            nc.vector.reduce_max(out=cf2[:, 1:2], in_=lmax8_sb[:, CPB:NCORES], axis=X)
            nc.vector.tensor_scalar_max(cf2, cf2, 0.0)

            # ---- router MLP for both batches (column layout [C, 2]);
            # bias+relu folded into one DVE tensor_scalar per layer
            h1p = pp.tile([C, B], f32)
            nc.tensor.matmul(out=h1p, lhsT=w1_sb, rhs=cf2, start=True, stop=True)
            h1 = sp.tile([C, B], f32)
            nc.vector.tensor_scalar(h1, h1p, b1_sb, 0.0, AT.add, AT.max)
            h2p = pp.tile([C, B], f32)
            nc.tensor.matmul(out=h2p, lhsT=w2_sb, rhs=h1, start=True, stop=True)
            h2 = sp.tile([C, B], f32)
            nc.vector.tensor_scalar(h2, h2p, b2_sb, 0.0, AT.add, AT.max)

            cp_pre = pp.tile([B, N], f32)
            nc.tensor.matmul(out=cp_pre, lhsT=h2, rhs=wp_sb, start=True, stop=True)
            ce_pre = pp.tile([B, N], f32)
            nc.tensor.matmul(out=ce_pre, lhsT=ipf2_sb, rhs=ce_mat, start=True, stop=True)

            eps2 = sp.tile([B, 1], f32)
            nc.vector.memset(eps2, LN_EPS)

            # ---- LayerNorm over the free axis of a [2, N] tile.
            # DVE-centric: single ACT visit (the sqrt), everything else DVE.
            def ln_rows(pre, post_scale, nm):
                pre_sb = sp.tile([B, N], f32, name=f"ln_pre_{nm}")
                nc.vector.tensor_copy(out=pre_sb, in_=pre)
                s = sp.tile([B, 1], f32, name=f"ln_s_{nm}")
                nc.vector.reduce_sum(out=s, in_=pre_sb, axis=X)
                sq = sp.tile([B, N], f32, name=f"ln_sq_{nm}")
                nc.vector.tensor_tensor(out=sq, in0=pre_sb, in1=pre_sb, op=AT.mult)
                s2 = sp.tile([B, 1], f32, name=f"ln_s2_{nm}")
                nc.vector.reduce_sum(out=s2, in_=sq, axis=X)
                m = sp.tile([B, 1], f32, name=f"ln_m_{nm}")
                nc.vector.tensor_scalar(m, s, 1.0 / N, None, AT.mult)
                xc = sp.tile([B, N], f32, name=f"ln_xc_{nm}")
                nc.vector.tensor_scalar(xc, pre_sb, m[:, 0:1], None, AT.subtract)
                m2 = sp.tile([B, 1], f32, name=f"ln_m2_{nm}")
                nc.vector.tensor_tensor(out=m2, in0=m, in1=m, op=AT.mult)
                v = sp.tile([B, 1], f32, name=f"ln_v_{nm}")
                nc.vector.tensor_scalar(v, s2, 1.0 / N, None, AT.mult)
                nc.vector.tensor_tensor(out=v, in0=v, in1=m2, op=AT.subtract)
                sd = sp.tile([B, 1], f32, name=f"ln_sd_{nm}")
                nc.scalar.activation(sd, v, Sqrt, bias=eps2[:, 0:1])
                rs = sp.tile([B, 1], f32, name=f"ln_rs_{nm}")
                nc.vector.reciprocal(rs, sd)
                if post_scale != 1.0:
                    nc.vector.tensor_scalar(rs, rs, post_scale, None, AT.mult)
                out = sp.tile([B, N], f32, name=f"ln_out_{nm}")
                nc.vector.tensor_scalar(out, xc, rs[:, 0:1], None, AT.mult)
                return out

            cp_row = ln_rows(cp_pre, 0.1, "cp")
            ce_row = ln_rows(ce_pre, 1.0, "ce")
            nc.scalar.dma_start(cp_out, cp_row)
            nc.scalar.dma_start(ce_out, ce_row)

            # ---- masked argmax -> one-hot selection for both batches
            logits = sp.tile([B, N], f32)
            nc.vector.tensor_add(logits, cp_row, ce_row)
            ml = sp.tile([B, N], f32)
            nc.vector.tensor_mul(ml, logits, cand2_sb)
            nc.vector.tensor_add(ml, ml, maskneg2_sb)
            mx2 = sp.tile([B, 1], f32)
            nc.vector.reduce_max(out=mx2, in_=ml, axis=X)
            sel2 = sp.tile([B, N], f32)
            nc.vector.tensor_scalar(sel2, ml, mx2[:, 0:1], None, AT.is_equal)
            nc.scalar.dma_start(sel_out, sel2)

            # ---- this core's selected camera index: nidx2 is host-masked to
            # this core's batch row, so a sum over both rows yields it.
            tsel = sp.tile([B, N], f32)
            nc.vector.tensor_mul(tsel, sel2, nidx2_sb)
            selv2 = sp.tile([B, 1], f32)
            nc.vector.reduce_sum(out=selv2, in_=tsel, axis=X)
            ones2 = sp.tile([B, 1], f32)
            nc.vector.memset(ones2, 1.0)
            svp = pp.tile([1, 1], f32)
            nc.tensor.matmul(out=svp, lhsT=selv2, rhs=ones2, start=True, stop=True)
            svi = sp.tile([1, 1], i32)
            nc.vector.tensor_copy(out=svi, in_=svp)
            r_sel = nc.values_load(
                svi[0:1, 0:1],
                engines=(mybir.EngineType.SP,),
                min_val=0,
                max_val=N - 1,
                skip_runtime_bounds_check=True,
            )

            # ---- gather selected camera (sync ring), combine (DVE),
            #      store (scalar ring)
            for t in range(NTD):
                sl = slice(BNDS[t], BNDS[t + 1])
                w = BNDS[t + 1] - BNDS[t]
                st = selp.tile([C, w], f32, name=f"selt{t}", tag="selt")
                nc.sync.dma_start(st, feat_blk[bass.ds(r_sel * C, C), sl])
                nc.vector.tensor_tensor(out=accs[t], in0=accs[t], in1=st, op=AT.max)
                nc.scalar.dma_start(out_blk[:, sl], accs[t])

    nc.compile()
    return nc


LAST_RESULTS = None
LAST_EXEC_NS = None


def kernel(**inputs):
    global _built1, _built2, LAST_RESULTS, LAST_EXEC_NS
    from concourse import bass_utils

    feat = np.ascontiguousarray(np.asarray(inputs["feat"], dtype=np.float32))
    init_prob = np.asarray(inputs["init_prob"]).astype(np.int64)
    keep_cams = np.asarray(inputs["keep_cams"])
    cam_emb = np.ascontiguousarray(np.asarray(inputs["cam_emb"], np.float32))
    W1 = np.asarray(inputs["W1"], np.float32)
    b1 = np.asarray(inputs["b1"], np.float32)
    W2 = np.asarray(inputs["W2"], np.float32)
    b2 = np.asarray(inputs["b2"], np.float32)
    Wp = np.asarray(inputs["Wp"], np.float32)

    if _built1 is None:
        _built1 = _build1()
    if _built2 is None:
        _built2 = _build2()

    fr = feat.reshape(B, N * C, HW)
    eye = np.eye(N, dtype=np.float32)
    ipf = eye[init_prob]                                 # [B, N]
    cand = (1.0 - ipf) * keep_cams.astype(np.float32)    # [B, N]

    blks = []
    inits = []
    for k in range(NCORES):
        b, q = divmod(k, CPB)
        ip = int(init_prob[b])
        blk = np.ascontiguousarray(fr[b][:, q * F : (q + 1) * F])
        blks.append(blk)
        inits.append(np.ascontiguousarray(blk[ip * C : (ip + 1) * C]))

    # ---- launch 1: per-core spatial max of the init chunk
    ident = np.eye(C, dtype=np.float32)
    in_maps1 = [{"feat_init": inits[k], "ident": ident} for k in range(NCORES)]
    res1 = bass_utils.run_bass_kernel_spmd(
        _built1, in_maps1, core_ids=list(range(NCORES))
    )

    # host relay (pure concatenation/layout, no arithmetic)
    lmax8 = np.ascontiguousarray(
        np.concatenate(
            [res1.results[k]["lmax_out"] for k in range(NCORES)], axis=0
        ).T
    )

    # ---- launch 2 (packed small inputs)
    big128 = np.ascontiguousarray(
        np.concatenate(
            [W1.T, W2.T, Wp.T, b1[:, None], b2[:, None], lmax8], axis=1
        ).astype(np.float32)
    )
    pk8 = np.ascontiguousarray(
        np.concatenate([cam_emb, ipf.T], axis=1).astype(np.float32)
    )
    maskneg = (cand - 1.0) * 1.0e30
    in_maps2 = []
    for k in range(NCORES):
        b, q = divmod(k, CPB)
        nidx2b = np.zeros((B, N), np.float32)
        nidx2b[b] = np.arange(N, dtype=np.float32)
        pk2 = np.ascontiguousarray(
            np.concatenate([cand, maskneg, nidx2b], axis=1).astype(np.float32)
        )
        in_maps2.append(
            {
                "big128": big128,
                "pk8": pk8,
                "pk2": pk2,
                "feat_blk": blks[k],
                "feat_init": inits[k],
            }
        )
    res2 = bass_utils.run_bass_kernel_spmd(
        _built2, in_maps2, core_ids=list(range(NCORES))
    )
    LAST_RESULTS = (res1, res2)
    if res1.exec_time_ns is not None and res2.exec_time_ns is not None:
        LAST_EXEC_NS = res1.exec_time_ns + res2.exec_time_ns
    outs = res2.results

    overall = np.empty((B, C, H, W), np.float32)
    for k in range(NCORES):
        b, q = divmod(k, CPB)
        overall[b, :, q * HC : (q + 1) * HC, :] = outs[k]["out_blk"].reshape(C, HC, W)
    ce = outs[0]["ce_out"]
    cp = outs[0]["cp_out"]
    sel = outs[0]["sel_out"]
    return overall, ce, cp, sel


# revision 33
# speedup vs baseline: 1.0804x; 1.0544x over previous
"""Trainium2 Bass kernel for CamPredModule (moe_routing) on 8 NeuronCores.

Reference semantics (eval path):
    ip        = one_hot(init_prob)                      # [B,N]
    init_feat = max(feat[b, ip_b], 0)                   # masked max over N
    ce        = layer_norm(cam_emb[ip_b])               # [B,N]
    cf        = relu(spatial_max(feat[b, ip_b]))        # [B,C]
    h         = relu(relu(cf@W1.T+b1)@W2.T+b2)
    cp        = layer_norm(h@Wp.T)/10
    sel       = argmax over candidates of (cp+ce)       # one-hot [B,N]
    overall   = max(init_feat, feat[b, sel_b])
    returns (overall, ce, cp, sel_onehot)

Sharding: core k handles (b = k//4, spatial chunk q = k%4 of H).  Each core
only touches the two needed camera slices (init + selected): the init slice
is host-sharded (a pure gather by the init_prob input index), the selected
slice is fetched with a dynamic-offset DMA using the on-device routing
result.

Two launches (the ncfw collective path has a ~60us fixed barrier on this
runtime, so the cross-core max is relayed through the host instead —
pure gather/concat of tiny [1,128] vectors, no host arithmetic):
  launch 1 (raw Bass, SP+DVE+PE): per-core spatial max of the init
      chunk, PE-transposed and stored as a contiguous row     (~32us)
  launch 2 (Tile): combine the 8 maxes, replicated router MLP for
      both batches, dynamic-offset gather of the selected camera,
      elementwise max against the relu'd init chunk, store    (~58us)
Per-core HBM traffic: launch 1 reads 5.5MB; launch 2 moves 16.5MB
(init re-read + gather + store) at the ~358GB/s/core roofline.
"""

import numpy as np

B, N, C, H, W = 2, 8, 128, 120, 360
HW = H * W          # 43200
NCORES = 8
CPB = 4             # cores (spatial chunks) per batch
F = HW // CPB       # 10800 elements per chunk per channel
NT = 3              # DMA/compute sub-tiles per chunk
F_TILE = F // NT    # 3600
HC = H // CPB       # 30 rows of H per chunk
LN_EPS = 1e-5

_built1 = None
_built2 = None


def _build1():
    """Launch 1: spatial max of the (host-gathered) init-camera chunk.

    Raw Bass (no TileContext): only the SP + DVE engines do work, manual
    semaphores, and a light tail — the Tile drain/cleanup epilogue costs
    ~15us, which matters at this kernel's ~25us scale.
    """
    import concourse.bass as bass
    import concourse.mybir as mybir

    f32 = mybir.dt.float32
    X = mybir.AxisListType.X
    # small first tile so the DVE reduce chain starts as early as possible;
    # the serial reduce work (~12us) then hides almost fully under the DMA
    BNDS1 = [0, 900, 4500, 8100, 10800]
    NT1 = len(BNDS1) - 1

    nc = bass.Bass("TRN2", target_bir_lowering=False, debug=False, num_devices=NCORES)
    feat_init = nc.dram_tensor("feat_init", [C, F], f32, kind="ExternalInput").ap()
    ident = nc.dram_tensor("ident", [C, C], f32, kind="ExternalInput").ap()
    # stored as a row: a [C,1] column store scatters 128 4-byte descriptors
    # (~8.5us); transposing on the PE and storing [1,C] contiguous is ~2us.
    lmax_out = nc.dram_tensor("lmax_out", [1, C], f32, kind="ExternalOutput").ap()

    # one semaphore per in-flight DMA: a shared counter is unsound because
    # SDMA engines drain their per-engine rings independently (a later DMA's
    # increments can land before an earlier DMA fully completes).
    dsems = [nc.alloc_semaphore(name=f"dsem{t}") for t in range(NT1)]
    isem = nc.alloc_semaphore(name="isem")
    ssem = nc.alloc_semaphore(name="ssem")
    vsem = nc.alloc_semaphore(name="vsem")
    psem = nc.alloc_semaphore(name="psem")
    msem = nc.alloc_semaphore(name="msem")
    tiles = [
        nc.alloc_sbuf_tensor(f"a{t}", [C, BNDS1[t + 1] - BNDS1[t]], f32).ap()
        for t in range(NT1)
    ]
    ident_sb = nc.alloc_sbuf_tensor("ident_sb", [C, C], f32).ap()
    pmax = nc.alloc_sbuf_tensor("pmax", [C, NT1], f32).ap()
    lmax = nc.alloc_sbuf_tensor("lmax", [C, 1], f32).ap()
    lrow = nc.alloc_sbuf_tensor("lrow", [1, C], f32).ap()
    lrow_ps = nc.alloc_psum_tensor("lrow_ps", [1, C], f32).ap()

    with nc.Block(no_gpsimd_drain=True) as block:

        @block.sync
        def _(sync):
            for t in range(NT1):
                sync.dma_start(
                    tiles[t], feat_init[:, BNDS1[t] : BNDS1[t + 1]]
                ).then_inc(dsems[t], 16)
            # needed only by the PE transpose at the very end
            sync.dma_start(ident_sb, ident).then_inc(isem, 16)
            sync.wait_ge(vsem, 2)
            sync.dma_start(lmax_out, lrow).then_inc(ssem, 16)
            sync.wait_ge(ssem, 16)

        @block.vector
        def _(vector):
            # DVE has no same-engine RAW interlock between back-to-back
            # instructions; the final reduce must wait for the partials'
            # writebacks via a self-semaphore.
            for t in range(NT1):
                vector.wait_ge(dsems[t], 16)
                vector.reduce_max(
                    out=pmax[:, t : t + 1], in_=tiles[t], axis=X
                ).then_inc(psem, 1)
            vector.wait_ge(psem, NT1)
            vector.reduce_max(out=lmax, in_=pmax, axis=X).then_inc(vsem, 1)
            vector.wait_ge(msem, 1)
            vector.tensor_copy(out=lrow, in_=lrow_ps).then_inc(vsem, 1)

        @block.tensor
        def _(tensor):
            tensor.wait_ge(isem, 16)
            tensor.wait_ge(vsem, 1)
            nc.tensor.matmul(
                out=lrow_ps, lhsT=lmax, rhs=ident_sb, start=True, stop=True
            ).then_inc(msem, 1)

    # reset semaphores so repeated executions of this NEFF start clean
    all_sems = sorted(s.num for s in (*dsems, isem, ssem, vsem, psem, msem))
    nc.gpsimd.sem_clear(range(min(all_sems), max(all_sems) + 1))
    return nc


def _build2():
    """Launch 2: combine maxes, router, dynamic gather, combine, store."""
    import concourse.bacc as bacc
    import concourse.bass as bass
    import concourse.mybir as mybir
    import concourse.tile as tile

    f32 = mybir.dt.float32
    i32 = mybir.dt.int32
    X = mybir.AxisListType.X
    Relu = mybir.ActivationFunctionType.Relu
    Sqrt = mybir.ActivationFunctionType.Sqrt
    AT = mybir.AluOpType

    nc = bacc.Bacc("TRN2", target_bir_lowering=False, debug=False, num_devices=NCORES)

    feat_blk = nc.dram_tensor("feat_blk", [N * C, F], f32, kind="ExternalInput").ap()
    feat_init = nc.dram_tensor("feat_init", [C, F], f32, kind="ExternalInput").ap()
    # packed small inputs (fewer DMAs -> router starts sooner):
    #   big128 columns: w1t | w2t | wpt | b1c | b2c | lmax8
    #   pk8 columns:    cam_emb | ipf2
    #   pk2 columns:    cand2 | maskneg2 | nidx2
    W128 = C + C + N + 1 + 1 + NCORES
    big128 = nc.dram_tensor("big128", [C, W128], f32, kind="ExternalInput").ap()
    pk8 = nc.dram_tensor("pk8", [N, N + B], f32, kind="ExternalInput").ap()
    pk2 = nc.dram_tensor("pk2", [B, 3 * N], f32, kind="ExternalInput").ap()

    out_blk = nc.dram_tensor("out_blk", [C, F], f32, kind="ExternalOutput").ap()
    ce_out = nc.dram_tensor("ce_out", [B, N], f32, kind="ExternalOutput").ap()
    cp_out = nc.dram_tensor("cp_out", [B, N], f32, kind="ExternalOutput").ap()
    sel_out = nc.dram_tensor("sel_out", [B, N], f32, kind="ExternalOutput").ap()

    with tile.TileContext(nc) as tc:
        with (
            tc.tile_pool(name="big", bufs=1) as bigp,
            tc.tile_pool(name="small", bufs=1) as sp,
            tc.tile_pool(name="selp", bufs=1) as selp,
            tc.tile_pool(name="psum", bufs=1, space="PSUM") as pp,
        ):
            # ---- packed small loads (scalar HWDGE ring)
            big_sb = sp.tile([C, W128], f32)
            nc.scalar.dma_start(big_sb, big128)
            pk8_sb = sp.tile([N, N + B], f32)
            nc.scalar.dma_start(pk8_sb, pk8)
            pk2_sb = sp.tile([B, 3 * N], f32)
            nc.scalar.dma_start(pk2_sb, pk2)
            w1_sb = big_sb[:, 0:C]
            w2_sb = big_sb[:, C : 2 * C]
            wp_sb = big_sb[:, 2 * C : 2 * C + N]
            b1_sb = big_sb[:, 2 * C + N : 2 * C + N + 1]
            b2_sb = big_sb[:, 2 * C + N + 1 : 2 * C + N + 2]
            lmax8_sb = big_sb[:, 2 * C + N + 2 : W128]
            ce_mat = pk8_sb[:, 0:N]
            ipf2_sb = pk8_sb[:, N : N + B]
            cand2_sb = pk2_sb[:, 0:N]
            maskneg2_sb = pk2_sb[:, N : 2 * N]
            nidx2_sb = pk2_sb[:, 2 * N : 3 * N]

            # ---- re-load the init chunk; clamp at 0 on the otherwise-idle
            # ACT engine while the router runs (overall = max(relu(init), sel)).
            # Descending tile sizes: the final gather->max->store chain in
            # phase D is short, trimming the serial tail after the last DMA.
            BNDS = [0, 3600, 7200, 9900, 10800]
            NTD = len(BNDS) - 1
            accs = []
            for t in range(NTD):
                sl = slice(BNDS[t], BNDS[t + 1])
                a = bigp.tile(
                    [C, BNDS[t + 1] - BNDS[t]], f32, name=f"acc{t}", tag=f"acc{t}"
                )
                accs.append(a)
                nc.sync.dma_start(a, feat_init[:, sl])
                nc.scalar.activation(a, a, Relu)

            # ---- global per-batch spatial max: columns 0-3 are batch 0's
            # chunks, 4-7 batch 1's (fixed core->(b,q) mapping)
            cf2 = sp.tile([C, B], f32)
            nc.vector.reduce_max(out=cf2[:, 0:1], in_=lmax8_sb[:, 0:CPB], axis=X)
            nc.vector.reduce_max(out=cf2[:, 1:2], in_=lmax8_sb[:, CPB:NCORES], axis=X)
            nc.vector.tensor_scalar_max(cf2, cf2, 0.0)

            # ---- router MLP for both batches (column layout [C, 2]);
            # bias+relu folded into one DVE tensor_scalar per layer
            h1p = pp.tile([C, B], f32)
            nc.tensor.matmul(out=h1p, lhsT=w1_sb, rhs=cf2, start=True, stop=True)
            h1 = sp.tile([C, B], f32)
            nc.vector.tensor_scalar(h1, h1p, b1_sb, 0.0, AT.add, AT.max)
            h2p = pp.tile([C, B], f32)
            nc.tensor.matmul(out=h2p, lhsT=w2_sb, rhs=h1, start=True, stop=True)
            h2 = sp.tile([C, B], f32)
            nc.vector.tensor_scalar(h2, h2p, b2_sb, 0.0, AT.add, AT.max)

            cp_pre = pp.tile([B, N], f32)
            nc.tensor.matmul(out=cp_pre, lhsT=h2, rhs=wp_sb, start=True, stop=True)
            ce_pre = pp.tile([B, N], f32)
            nc.tensor.matmul(out=ce_pre, lhsT=ipf2_sb, rhs=ce_mat, start=True, stop=True)

            eps2 = sp.tile([B, 1], f32)
            nc.vector.memset(eps2, LN_EPS)

            # ---- LayerNorm over the free axis of a [2, N] tile.
            # DVE-centric: single ACT visit (the sqrt), everything else DVE.
            def ln_rows(pre, post_scale, nm):
                pre_sb = sp.tile([B, N], f32, name=f"ln_pre_{nm}")
                nc.vector.tensor_copy(out=pre_sb, in_=pre)
                s = sp.tile([B, 1], f32, name=f"ln_s_{nm}")
                nc.vector.reduce_sum(out=s, in_=pre_sb, axis=X)
                sq = sp.tile([B, N], f32, name=f"ln_sq_{nm}")
                nc.vector.tensor_tensor(out=sq, in0=pre_sb, in1=pre_sb, op=AT.mult)
                s2 = sp.tile([B, 1], f32, name=f"ln_s2_{nm}")
                nc.vector.reduce_sum(out=s2, in_=sq, axis=X)
                m = sp.tile([B, 1], f32, name=f"ln_m_{nm}")
                nc.vector.tensor_scalar(m, s, 1.0 / N, None, AT.mult)
                xc = sp.tile([B, N], f32, name=f"ln_xc_{nm}")
                nc.vector.tensor_scalar(xc, pre_sb, m[:, 0:1], None, AT.subtract)
                m2 = sp.tile([B, 1], f32, name=f"ln_m2_{nm}")
                nc.vector.tensor_tensor(out=m2, in0=m, in1=m, op=AT.mult)
                v = sp.tile([B, 1], f32, name=f"ln_v_{nm}")
                nc.vector.tensor_scalar(v, s2, 1.0 / N, None, AT.mult)
                nc.vector.tensor_tensor(out=v, in0=v, in1=m2, op=AT.subtract)
                sd = sp.tile([B, 1], f32, name=f"ln_sd_{nm}")
                nc.scalar.activation(sd, v, Sqrt, bias=eps2[:, 0:1])
                rs = sp.tile([B, 1], f32, name=f"ln_rs_{nm}")
                nc.vector.reciprocal(rs, sd)
                if post_scale != 1.0:
                    nc.vector.tensor_scalar(rs, rs, post_scale, None, AT.mult)
                out = sp.tile([B, N], f32, name=f"ln_out_{nm}")
                nc.vector.tensor_scalar(out, xc, rs[:, 0:1], None, AT.mult)
                return out

            cp_row = ln_rows(cp_pre, 0.1, "cp")
            ce_row = ln_rows(ce_pre, 1.0, "ce")
            nc.scalar.dma_start(cp_out, cp_row)
            nc.scalar.dma_start(ce_out, ce_row)

            # ---- masked argmax -> one-hot selection for both batches
            logits = sp.tile([B, N], f32)
            nc.vector.tensor_add(logits, cp_row, ce_row)
            ml = sp.tile([B, N], f32)
            nc.vector.tensor_mul(ml, logits, cand2_sb)
            nc.vector.tensor_add(ml, ml, maskneg2_sb)
            mx2 = sp.tile([B, 1], f32)
            nc.vector.reduce_max(out=mx2, in_=ml, axis=X)
            sel2 = sp.tile([B, N], f32)
            nc.vector.tensor_scalar(sel2, ml, mx2[:, 0:1], None, AT.is_equal)
            nc.scalar.dma_start(sel_out, sel2)

            # ---- this core's selected camera index: nidx2 is host-masked to
            # this core's batch row, so a sum over both rows yields it.
            tsel = sp.tile([B, N], f32)
            nc.vector.tensor_mul(tsel, sel2, nidx2_sb)
            selv2 = sp.tile([B, 1], f32)
            nc.vector.reduce_sum(out=selv2, in_=tsel, axis=X)
            ones2 = sp.tile([B, 1], f32)
            nc.vector.memset(ones2, 1.0)
            svp = pp.tile([1, 1], f32)
            nc.tensor.matmul(out=svp, lhsT=selv2, rhs=ones2, start=True, stop=True)
            svi = sp.tile([1, 1], i32)
            nc.vector.tensor_copy(out=svi, in_=svp)
            r_sel = nc.values_load(
                svi[0:1, 0:1],
                engines=(mybir.EngineType.SP,),
                min_val=0,
                max_val=N - 1,
                skip_runtime_bounds_check=True,
            )

            # ---- gather selected camera (sync ring), combine (DVE),
            #      store (scalar ring)
            for t in range(NTD):
                sl = slice(BNDS[t], BNDS[t + 1])
                w = BNDS[t + 1] - BNDS[t]
                # distinct tags: each gather gets its own buffer, so a later
                # gather never stalls on an earlier tile's max releasing a slot
                st = selp.tile([C, w], f32, name=f"selt{t}", tag=f"selt{t}")
                nc.sync.dma_start(st, feat_blk[bass.ds(r_sel * C, C), sl])
                nc.vector.tensor_tensor(out=accs[t], in0=accs[t], in1=st, op=AT.max)
                nc.scalar.dma_start(out_blk[:, sl], accs[t])

    nc.compile()
    return nc


LAST_RESULTS = None
LAST_EXEC_NS = None


def kernel(**inputs):
    global _built1, _built2, LAST_RESULTS, LAST_EXEC_NS
    from concourse import bass_utils

    feat = np.ascontiguousarray(np.asarray(inputs["feat"], dtype=np.float32))
    init_prob = np.asarray(inputs["init_prob"]).astype(np.int64)
    keep_cams = np.asarray(inputs["keep_cams"])
    cam_emb = np.ascontiguousarray(np.asarray(inputs["cam_emb"], np.float32))
    W1 = np.asarray(inputs["W1"], np.float32)
    b1 = np.asarray(inputs["b1"], np.float32)
    W2 = np.asarray(inputs["W2"], np.float32)
    b2 = np.asarray(inputs["b2"], np.float32)
    Wp = np.asarray(inputs["Wp"], np.float32)

    if _built1 is None:
        _built1 = _build1()
    if _built2 is None:
        _built2 = _build2()

    fr = feat.reshape(B, N * C, HW)
    eye = np.eye(N, dtype=np.float32)
    ipf = eye[init_prob]                                 # [B, N]
    cand = (1.0 - ipf) * keep_cams.astype(np.float32)    # [B, N]

    blks = []
    inits = []
    for k in range(NCORES):
        b, q = divmod(k, CPB)
        ip = int(init_prob[b])
        blk = np.ascontiguousarray(fr[b][:, q * F : (q + 1) * F])
        blks.append(blk)
        inits.append(np.ascontiguousarray(blk[ip * C : (ip + 1) * C]))

    # ---- launch 1: per-core spatial max of the init chunk
    ident = np.eye(C, dtype=np.float32)
    in_maps1 = [{"feat_init": inits[k], "ident": ident} for k in range(NCORES)]
    res1 = bass_utils.run_bass_kernel_spmd(
        _built1, in_maps1, core_ids=list(range(NCORES))
    )

    # host relay (pure concatenation/layout, no arithmetic)
    lmax8 = np.ascontiguousarray(
        np.concatenate(
            [res1.results[k]["lmax_out"] for k in range(NCORES)], axis=0
        ).T
    )

    # ---- launch 2 (packed small inputs)
    big128 = np.ascontiguousarray(
        np.concatenate(
            [W1.T, W2.T, Wp.T, b1[:, None], b2[:, None], lmax8], axis=1
        ).astype(np.float32)
    )
    pk8 = np.ascontiguousarray(
        np.concatenate([cam_emb, ipf.T], axis=1).astype(np.float32)
    )
    maskneg = (cand - 1.0) * 1.0e30
    in_maps2 = []
    for k in range(NCORES):
        b, q = divmod(k, CPB)
        nidx2b = np.zeros((B, N), np.float32)
        nidx2b[b] = np.arange(N, dtype=np.float32)
        pk2 = np.ascontiguousarray(
            np.concatenate([cand, maskneg, nidx2b], axis=1).astype(np.float32)
        )
        in_maps2.append(
            {
                "big128": big128,
                "pk8": pk8,
                "pk2": pk2,
                "feat_blk": blks[k],
                "feat_init": inits[k],
            }
        )
    res2 = bass_utils.run_bass_kernel_spmd(
        _built2, in_maps2, core_ids=list(range(NCORES))
    )
    LAST_RESULTS = (res1, res2)
    if res1.exec_time_ns is not None and res2.exec_time_ns is not None:
        LAST_EXEC_NS = res1.exec_time_ns + res2.exec_time_ns
    outs = res2.results

    overall = np.empty((B, C, H, W), np.float32)
    for k in range(NCORES):
        b, q = divmod(k, CPB)
        overall[b, :, q * HC : (q + 1) * HC, :] = outs[k]["out_blk"].reshape(C, HC, W)
    ce = outs[0]["ce_out"]
    cp = outs[0]["cp_out"]
    sel = outs[0]["sel_out"]
    return overall, ce, cp, sel
